# revision 43
# baseline (speedup 1.0000x reference)
"""Trainium2 Bass kernel: masked multi-head decode attention + output projection.

Problem (hardcoded): query [256,1,512] f32, key/value [256,2048,512] f32,
W_o [512,512] f32, mask [256,1,2048] bool (True = excluded).
out = Linear(W_o) o MHA(query, key, value, mask), 8 heads, dh=64.

Strategy: data-parallel over batch on 8 NeuronCores (32 batches/core).
~50% of keys are masked out and contribute nothing, so the host packs only
the unmasked K/V rows per batch (padded to KPAD=1152 = 9 tiles of 128) and
stores them in bf16 — ~3.5x less HBM traffic than the dense f32 layout.

Per batch on-core:
  - K arrives pre-transposed: kt[p, c, j] = K[packed key j, dim c*128+p] so
    scores come from the PE: s[j-tile, h] = sum_c kt_chunk^T @ qblk_c where
    qblk is the block-diagonal q/8 (only head h(d) column is nonzero), i.e.
    4 accumulating [128d x 128k x 8h] matmuls per 128-key tile.
  - a = exp(s - 3) on ACT (global shift keeps exp small; softmax cancels it).
    Padded slots j >= count get a host-built poison K column with
    poison_d = -240*q_d/||q_head(d)||^2 so s_pad = -30 exactly -> a ~ 0.
  - merged[h, e] += a_tile^T @ V_tile and denom[h] += a_tile^T @ ones as
    accumulating matmuls over the 9 key tiles (V natural layout, bf16).
  - normalize on DVE (reciprocal + tensor_scalar_mul), head-diagonal extract
    via 8 one-hot matmuls into a persistent merged^T PSUM tile.
Tail (once per core): out[32, 512] = sum_c mT_c^T @ W_o^T chunk on PE.
"""

import numpy as np

N_CORES = 8
BATCH = 256
NKEYS = 2048
EMB = 512
NH = 8
DH = 64
P = 128
KPAD = 1152          # padded packed-key count, 9 tiles of 128 (max count 1095)
NT = KPAD // P       # 9 key tiles per batch
NCH = EMB // P       # 4 contraction chunks over embedding dims
B_LOC = BATCH // N_CORES  # 32
NB9 = 18             # per-core batch positions 0..17 have 9 key tiles, rest 8
QSCALE = 1.0 / 8.0   # 1/sqrt(dh)
EXP_SHIFT = -3.0     # a = exp(s - 3); cancels in softmax, keeps a in bf16 range
POISON_S = -30.0     # target score for padded key slots


def build_nc(nb=B_LOC):
    """Build + compile the Bass program for one core processing `nb` batches."""
    import concourse.bass as bass
    import concourse.tile as tile
    from concourse import bacc, mybir

    f32 = mybir.dt.float32
    bf16 = mybir.dt.bfloat16
    f8e3 = mybir.dt.float8e3

    nc = bacc.Bacc(
        "TRN2",
        target_bir_lowering=False,
        debug=False,
        enable_asserts=True,
        num_devices=N_CORES,
    )
    kpt = nc.dram_tensor("kpt", [P, nb, NT, NCH, P], bf16, kind="ExternalInput").ap()
    vp = nc.dram_tensor("vp", [P, nb, NT, EMB], f8e3, kind="ExternalInput").ap()
    qblk = nc.dram_tensor("qblk", [P, nb, NCH, NH], bf16, kind="ExternalInput").ap()
    wot = nc.dram_tensor("wot", [EMB, EMB], f32, kind="ExternalInput").ap()
    onesd = nc.dram_tensor("ones", [P, 1], bf16, kind="ExternalInput").ap()
    seld = nc.dram_tensor("sel", [NH, NCH, P], f32, kind="ExternalInput").ap()
    out = nc.dram_tensor("out", [nb, EMB], f32, kind="ExternalOutput").ap()
    import os

    dbg = None
    if os.environ.get("KERNEL_DEBUG"):
        dbg = {
            "mt": nc.dram_tensor("dbg_mt", [P, NCH, nb], f32, kind="ExternalOutput").ap(),
            "rall": nc.dram_tensor("dbg_rall", [NH, nb], f32, kind="ExternalOutput").ap(),
            "scl": nc.dram_tensor("dbg_scl", [P, NCH, nb], f32, kind="ExternalOutput").ap(),
        }

    with tile.TileContext(nc) as tc:
        _emit(tc, out, kpt, vp, qblk, wot, onesd, seld, nb, dbg)
    nc.compile()
    return nc


def _emit(tc, out, kpt, vp, qblk, wot, onesd, seld, nb, dbg=None):
    from contextlib import ExitStack

    from concourse import mybir

    f32 = mybir.dt.float32
    bf16 = mybir.dt.bfloat16
    f8e3 = mybir.dt.float8e3
    nc = tc.nc

    with ExitStack() as ctx:
        kpool = ctx.enter_context(tc.tile_pool(name="kpool", bufs=8))
        vpool = ctx.enter_context(tc.tile_pool(name="vpool", bufs=8))
        apool = ctx.enter_context(tc.tile_pool(name="apool", bufs=6))
        sp = ctx.enter_context(tc.tile_pool(name="sp", bufs=4))
        singles = ctx.enter_context(tc.tile_pool(name="singles", bufs=1))
        psum_s = ctx.enter_context(tc.tile_pool(name="psum_s", bufs=2, space="PSUM"))
        psum_mcs = [
            ctx.enter_context(tc.tile_pool(name=f"psum_mc{c}", bufs=1, space="PSUM"))
            for c in range(NCH)
        ]
        psum_d = ctx.enter_context(tc.tile_pool(name="psum_d", bufs=1, space="PSUM"))
        psum_o = ctx.enter_context(tc.tile_pool(name="psum_o", bufs=1, space="PSUM"))

        ones_sb = singles.tile([P, 1], bf16)
        nc.sync.dma_start(ones_sb[:], onesd)
        ebias_sb = singles.tile([P, 1], f32)
        nc.gpsimd.memset(ebias_sb[:], EXP_SHIFT)
        qblk_sb = singles.tile([P, nb, NCH, NH], bf16)
        nc.sync.dma_start(qblk_sb[:], qblk)
        # wot/sel are only needed for the final projection: emitted after the
        # batch loop so their DMAs don't delay the first K/V transfers.
        wot_sb = singles.tile([P, NCH, EMB], f32)
        sel_sb = singles.tile([NH, NCH, P], f32)
        # unnormalized merged^T, built per batch by the extract copies:
        # mt_sb[p, c, b] = sum_k a_b[k, h] V_b[k, c*128+p], h = 2c + p//64
        mt_sb = singles.tile([P, NCH, nb], f32)
        # per-batch denominators, accumulated per column: dall[h, b]
        dall_ps = psum_d.tile([NH, nb], f32)

        for b in range(nb):
            # batches at positions >= NB9 have <= 1024 packed keys: skip the
            # all-padding 9th key tile entirely (less HBM traffic + PE work).
            ntb = NT if b < NB9 else NT - 1
            kt = kpool.tile([P, ntb, NCH, P], bf16, tag="k")
            nc.sync.dma_start(kt[:], kpt[:, b, 0:ntb])
            vt = vpool.tile([P, ntb, EMB], f8e3, tag="v")
            nc.scalar.dma_start(vt[:], vp[:, b, 0:ntb, :])
            if b == 2:
                nc.sync.dma_start(wot_sb[:], wot.rearrange("(c p) e -> p c e", p=P))
                nc.sync.dma_start(sel_sb[:], seld)

            # merged^T chunk accumulators [128 dims, 8 heads], one per e-chunk
            mc = [
                psum_mcs[c].tile([P, NH], f32, name=f"mc{c}", tag="mc")
                for c in range(NCH)
            ]

            def _merged(j, a_j):
                for c in range(NCH):
                    nc.tensor.matmul(
                        mc[c][:],
                        vt[:, j, c * P : (c + 1) * P],
                        a_j[:],
                        start=(j == 0),
                        stop=(j == ntb - 1),
                    )
                nc.tensor.matmul(
                    dall_ps[:, b : b + 1],
                    a_j[:],
                    ones_sb[:],
                    start=(j == 0),
                    stop=(j == ntb - 1),
                )

            # scores (PE) + exp (ACT), with merged/denom lagging two tiles so
            # the PE never waits on the ACT exp.
            lag = []
            for j in range(ntb):
                s_ps = psum_s.tile([P, NH], f32, tag="sps")
                for c in range(NCH):
                    nc.tensor.matmul(
                        s_ps[:],
                        kt[:, j, c, :],
                        qblk_sb[:, b, c, :],
                        start=(c == 0),
                        stop=(c == NCH - 1),
                    )
                a_j = apool.tile([P, NH], bf16, tag="a")
                nc.scalar.activation(
                    a_j[:],
                    s_ps[:],
                    mybir.ActivationFunctionType.Exp,
                    bias=ebias_sb[:],
                )
                lag.append((j, a_j))
                if len(lag) > 2:
                    _merged(*lag.pop(0))
            while lag:
                _merged(*lag.pop(0))

            # extract the head-diagonal into SBUF: mt_sb[hp*64+p', c, b] =
            # mc[c][hp*64+p', 2c+hp]; split copies across ACT and DVE.
            for c in range(NCH):
                for hp in range(2):
                    h = 2 * c + hp
                    src = mc[c][hp * DH : (hp + 1) * DH, h : h + 1]
                    dst = mt_sb[hp * DH : (hp + 1) * DH, c, b : b + 1]
                    if h % 2 == 0:
                        nc.scalar.copy(dst, src)
                    else:
                        nc.vector.tensor_copy(dst, src)

        # ---- tail: normalize by 1/denom, project through W_o^T
        rall = sp.tile([NH, nb], f32, tag="rall")
        nc.vector.reciprocal(rall[:], dall_ps[:])
        # broadcast rall rows down the partitions via a PE one-hot select:
        # scaleT[p, c, b] = sum_h sel[h, c, p] * rall[h, b] = rall[h(p,c), b]
        scaleT_ps = psum_mcs[0].tile([P, NCH, nb], f32, name="scl", tag="mc")
        for c in range(NCH):
            nc.tensor.matmul(
                scaleT_ps[:, c, :],
                sel_sb[:, c, :],
                rall[:],
                start=True,
                stop=True,
            )
        mt_n = singles.tile([P, NCH, nb], f32)
        nc.vector.tensor_mul(mt_n[:], mt_sb[:], scaleT_ps[:])
        if dbg is not None:
            nc.sync.dma_start(dbg["mt"], mt_sb[:])
            nc.sync.dma_start(dbg["rall"], rall[:])
            scl_sb = singles.tile([P, NCH, nb], f32)
            nc.vector.tensor_copy(scl_sb[:], scaleT_ps[:])
            nc.sync.dma_start(dbg["scl"], scl_sb[:])
        out_ps = psum_o.tile([nb, EMB], f32, tag="ops")
        for c in range(NCH):
            nc.tensor.matmul(
                out_ps[:],
                mt_n[:, c, :],
                wot_sb[:, c, :],
                start=(c == 0),
                stop=(c == NCH - 1),
            )
        out_sb = singles.tile([nb, EMB], f32)
        nc.vector.tensor_copy(out_sb[:], out_ps[:])
        nc.sync.dma_start(out, out_sb[:])


def _prep_all(query, key, value, W_o, mask):
    """Host-side pack: gather unmasked K/V rows, pad, quantize, per-core shards.

    Returns (shards, perm): batches are permuted so every core's positions
    0..NB9-1 hold the batches with >1024 unmasked keys (9 key tiles); the rest
    need only 8. perm[i] = original batch index at packed position i.
    """
    import ml_dtypes

    bf16 = ml_dtypes.bfloat16
    f8e3 = ml_dtypes.float8_e3m4
    q_all = np.ascontiguousarray(query[:, 0, :], dtype=np.float32)  # [B, E]
    keep_all = ~mask[:, 0, :]  # True = attended
    counts_all = keep_all.sum(1).astype(np.int64)
    assert counts_all.max() <= KPAD, f"KPAD too small: {counts_all.max()}"

    heavy = np.flatnonzero(counts_all > (NT - 1) * P)
    light = np.flatnonzero(counts_all <= (NT - 1) * P)
    assert len(heavy) <= N_CORES * NB9, f"too many 9-tile batches: {len(heavy)}"
    pool9 = list(heavy)
    pool8 = list(light)
    per_core = [[] for _ in range(N_CORES)]
    for c in range(N_CORES):
        for _ in range(NB9):
            per_core[c].append(pool9.pop(0) if pool9 else pool8.pop(0))
    for c in range(N_CORES):
        for _ in range(B_LOC - NB9):
            per_core[c].append(pool8.pop(0))
    assert not pool9 and not pool8
    perm = np.array([b for core in per_core for b in core], dtype=np.int64)

    q = q_all[perm]
    keep = keep_all[perm]
    counts = counts_all[perm]

    gidx = np.empty((BATCH, KPAD), dtype=np.int64)
    for i in range(BATCH):
        idx = np.flatnonzero(keep[i])
        n = len(idx)
        gidx[i, :n] = idx
        gidx[i, n:] = idx[0] if n else 0
    rows = (gidx + perm[:, None] * NKEYS).reshape(-1)

    # K: gather packed rows, poison the padded slots, bf16, transpose to
    # [P, B, NCH, KPAD] with kpt[p, b, c, j] = Kp[b, j, c*128+p].
    Kp = key.reshape(-1, EMB)[rows].reshape(BATCH, KPAD, EMB)
    qh2 = (q.reshape(BATCH, NH, DH) ** 2).sum(-1)  # [B, H] per-head |q|^2
    poison = (POISON_S / QSCALE) * q / np.repeat(qh2, DH, axis=1)  # [B, E]
    padpos = np.arange(KPAD)[None, :] >= counts[:, None]
    bi, ji = np.nonzero(padpos)
    Kp[bi, ji] = poison[bi]
    # tile-major: kpt[p, b, t, c, j2] = Kp[b, t*128+j2, c*128+p] so any
    # leading-tile slice is one contiguous per-partition DMA.
    kpt = np.ascontiguousarray(
        Kp.astype(bf16).reshape(BATCH, NT, P, NCH, P).transpose(4, 0, 1, 3, 2)
    )

    # V: gather packed rows (pad rows harmless: their weight is exp(-30)),
    # fp8 e3m4 (max 15.5 >> |V|max ~5.5; rel err ~1.8% washes out in the
    # weighted sum), [P, B, NT, EMB] with vp[p, b, t, e] = Vp[b, t*128+p, e].
    Vp = value.reshape(-1, EMB)[rows].reshape(BATCH, KPAD, EMB)
    vp = np.ascontiguousarray(
        Vp.astype(f8e3).reshape(BATCH, NT, P, EMB).transpose(2, 0, 1, 3)
    )

    # qblk [P, B, NCH, NH]: block-diagonal scaled q. Column h = 2c + (p>=64)
    # holds q[c*128+p]/8; all other columns zero.
    qs = (q * np.float32(QSCALE)).reshape(BATCH, NCH, 2, DH)
    qblk = np.zeros((BATCH, NCH, P, NH), np.float32)
    for c in range(NCH):
        for hp in range(2):
            qblk[:, c, hp * DH : (hp + 1) * DH, 2 * c + hp] = qs[:, c, hp, :]
    qblk = np.ascontiguousarray(qblk.transpose(2, 0, 1, 3).astype(bf16))

    wot = np.ascontiguousarray(W_o.T).astype(np.float32)
    ones = np.ones((P, 1), dtype=bf16)
    sel = np.zeros((NH, NCH, P), dtype=np.float32)
    for c in range(NCH):
        for hp in range(2):
            sel[2 * c + hp, c, hp * DH : (hp + 1) * DH] = 1.0

    shards = []
    for c in range(N_CORES):
        lo, hi = c * B_LOC, (c + 1) * B_LOC
        shards.append(
            {
                "kpt": np.ascontiguousarray(kpt[:, lo:hi]),
                "vp": np.ascontiguousarray(vp[:, lo:hi]),
                "qblk": np.ascontiguousarray(qblk[:, lo:hi]),
                "wot": wot,
                "ones": ones,
                "sel": sel,
            }
        )
    return shards, perm


_NC_CACHE = {}


def _get_nc():
    if "nc" not in _NC_CACHE:
        _NC_CACHE["nc"] = build_nc()
    return _NC_CACHE["nc"]


def kernel(query, key, value, W_o, mask):
    from concourse import bass_utils

    query = np.asarray(query, dtype=np.float32)
    key = np.asarray(key, dtype=np.float32)
    value = np.asarray(value, dtype=np.float32)
    W_o = np.asarray(W_o, dtype=np.float32)
    mask = np.asarray(mask)

    nc = _get_nc()
    in_maps, perm = _prep_all(query, key, value, W_o, mask)
    res = bass_utils.run_bass_kernel_spmd(
        nc, in_maps, core_ids=list(range(N_CORES)), trace=False
    )
    out_perm = np.concatenate([res.results[c]["out"] for c in range(N_CORES)], axis=0)
    out = np.empty_like(out_perm)
    out[perm] = out_perm
    return out.reshape(BATCH, 1, EMB).astype(np.float32, copy=False)


if __name__ == "__main__":
    # smoke: build the program only
    nc = build_nc()
    print("built + compiled OK; instructions:", len(list(nc.all_instructions())))


# revision 46
# speedup vs baseline: 1.0689x; 1.0689x over previous
"""Trainium2 Bass kernel: masked multi-head decode attention + output projection.

Problem (hardcoded): query [256,1,512] f32, key/value [256,2048,512] f32,
W_o [512,512] f32, mask [256,1,2048] bool (True = excluded).
out = Linear(W_o) o MHA(query, key, value, mask), 8 heads, dh=64.

Strategy: data-parallel over batch on 8 NeuronCores (32 batches/core).
~50% of keys are masked out and contribute nothing, so the host packs only
the unmasked K/V rows per batch (padded to KPAD=1152 = 9 tiles of 128) and
stores them in bf16 — ~3.5x less HBM traffic than the dense f32 layout.

Per batch on-core:
  - K arrives pre-transposed: kt[p, c, j] = K[packed key j, dim c*128+p] so
    scores come from the PE: s[j-tile, h] = sum_c kt_chunk^T @ qblk_c where
    qblk is the block-diagonal q/8 (only head h(d) column is nonzero), i.e.
    4 accumulating [128d x 128k x 8h] matmuls per 128-key tile.
  - a = exp(s - 3) on ACT (global shift keeps exp small; softmax cancels it).
    Padded slots j >= count get a host-built poison K column with
    poison_d = -240*q_d/||q_head(d)||^2 so s_pad = -30 exactly -> a ~ 0.
  - merged[h, e] += a_tile^T @ V_tile and denom[h] += a_tile^T @ ones as
    accumulating matmuls over the 9 key tiles (V natural layout, bf16).
  - normalize on DVE (reciprocal + tensor_scalar_mul), head-diagonal extract
    via 8 one-hot matmuls into a persistent merged^T PSUM tile.
Tail (once per core): out[32, 512] = sum_c mT_c^T @ W_o^T chunk on PE.
"""

import numpy as np

N_CORES = 8
BATCH = 256
NKEYS = 2048
EMB = 512
NH = 8
DH = 64
P = 128
KPAD = 1152          # padded packed-key count, 9 tiles of 128 (max count 1095)
NT = KPAD // P       # 9 key tiles per batch
NCH = EMB // P       # 4 contraction chunks over embedding dims
B_LOC = BATCH // N_CORES  # 32
NB9 = 18             # per-core batch positions 0..17 have 9 key tiles, rest 8
QSCALE = 1.0 / 8.0   # 1/sqrt(dh)
EXP_SHIFT = -3.0     # a = exp(s - 3); cancels in softmax, keeps a in bf16 range
POISON_S = -30.0     # target score for padded key slots


def build_nc(nb=B_LOC):
    """Build + compile the Bass program for one core processing `nb` batches."""
    import concourse.bass as bass
    import concourse.tile as tile
    from concourse import bacc, mybir

    f32 = mybir.dt.float32
    bf16 = mybir.dt.bfloat16
    f8e3 = mybir.dt.float8e3

    nc = bacc.Bacc(
        "TRN2",
        target_bir_lowering=False,
        debug=False,
        enable_asserts=True,
        num_devices=N_CORES,
    )
    kpt = nc.dram_tensor("kpt", [P, nb, NT, NCH, P], bf16, kind="ExternalInput").ap()
    vp = nc.dram_tensor("vp", [P, nb, NT, EMB], f8e3, kind="ExternalInput").ap()
    qblk = nc.dram_tensor("qblk", [P, nb, NCH, NH], bf16, kind="ExternalInput").ap()
    wot = nc.dram_tensor("wot", [EMB, EMB], f32, kind="ExternalInput").ap()
    onesd = nc.dram_tensor("ones", [P, 1], bf16, kind="ExternalInput").ap()
    seld = nc.dram_tensor("sel", [NH, NCH, P], f32, kind="ExternalInput").ap()
    out = nc.dram_tensor("out", [nb, EMB], f32, kind="ExternalOutput").ap()
    import os

    dbg = None
    if os.environ.get("KERNEL_DEBUG"):
        dbg = {
            "mt": nc.dram_tensor("dbg_mt", [P, NCH, nb], f32, kind="ExternalOutput").ap(),
            "rall": nc.dram_tensor("dbg_rall", [NH, nb], f32, kind="ExternalOutput").ap(),
            "scl": nc.dram_tensor("dbg_scl", [P, NCH, nb], f32, kind="ExternalOutput").ap(),
        }

    with tile.TileContext(nc) as tc:
        _emit(tc, out, kpt, vp, qblk, wot, onesd, seld, nb, dbg)
    nc.compile()
    return nc


def _emit(tc, out, kpt, vp, qblk, wot, onesd, seld, nb, dbg=None):
    from contextlib import ExitStack

    from concourse import mybir

    f32 = mybir.dt.float32
    bf16 = mybir.dt.bfloat16
    f8e3 = mybir.dt.float8e3
    nc = tc.nc

    with ExitStack() as ctx:
        kpool = ctx.enter_context(tc.tile_pool(name="kpool", bufs=6))
        vpool = ctx.enter_context(tc.tile_pool(name="vpool", bufs=6))
        apool = ctx.enter_context(tc.tile_pool(name="apool", bufs=6))
        sp = ctx.enter_context(tc.tile_pool(name="sp", bufs=4))
        singles = ctx.enter_context(tc.tile_pool(name="singles", bufs=1))
        psum_s = ctx.enter_context(tc.tile_pool(name="psum_s", bufs=2, space="PSUM"))
        psum_mcs = [
            ctx.enter_context(tc.tile_pool(name=f"psum_mc{c}", bufs=1, space="PSUM"))
            for c in range(NCH)
        ]
        psum_d = ctx.enter_context(tc.tile_pool(name="psum_d", bufs=1, space="PSUM"))
        psum_o = ctx.enter_context(tc.tile_pool(name="psum_o", bufs=1, space="PSUM"))

        ones_sb = singles.tile([P, 1], bf16)
        nc.sync.dma_start(ones_sb[:], onesd)
        ebias_sb = singles.tile([P, 1], f32)
        nc.gpsimd.memset(ebias_sb[:], EXP_SHIFT)
        qblk_sb = singles.tile([P, nb, NCH, NH], bf16)
        nc.sync.dma_start(qblk_sb[:], qblk)
        wot_sb = singles.tile([P, NCH, EMB], f32)
        nc.sync.dma_start(wot_sb[:], wot.rearrange("(c p) e -> p c e", p=P))
        sel_sb = singles.tile([NH, NCH, P], f32)
        nc.sync.dma_start(sel_sb[:], seld)
        # unnormalized merged^T, built per batch by the extract copies:
        # mt_sb[p, c, b] = sum_k a_b[k, h] V_b[k, c*128+p], h = 2c + p//64
        mt_sb = singles.tile([P, NCH, nb], f32)
        # per-batch denominators, accumulated per column: dall[h, b]
        dall_ps = psum_d.tile([NH, nb], f32)

        for b in range(nb):
            # batches at positions >= NB9 have <= 1024 packed keys: skip the
            # all-padding 9th key tile entirely (less HBM traffic + PE work).
            ntb = NT if b < NB9 else NT - 1
            kt = kpool.tile([P, ntb, NCH, P], bf16, tag="k")
            nc.sync.dma_start(kt[:], kpt[:, b, 0:ntb])
            vt = vpool.tile([P, ntb, EMB], f8e3, tag="v")
            nc.scalar.dma_start(vt[:], vp[:, b, 0:ntb, :])

            # merged^T chunk accumulators [128 dims, 8 heads], one per e-chunk
            mc = [
                psum_mcs[c].tile([P, NH], f32, name=f"mc{c}", tag="mc")
                for c in range(NCH)
            ]

            def _merged(j, a_j):
                for c in range(NCH):
                    nc.tensor.matmul(
                        mc[c][:],
                        vt[:, j, c * P : (c + 1) * P],
                        a_j[:],
                        start=(j == 0),
                        stop=(j == ntb - 1),
                    )
                nc.tensor.matmul(
                    dall_ps[:, b : b + 1],
                    a_j[:],
                    ones_sb[:],
                    start=(j == 0),
                    stop=(j == ntb - 1),
                )

            # scores (PE) + exp (ACT), with merged/denom lagging two tiles so
            # the PE never waits on the ACT exp.
            lag = []
            for j in range(ntb):
                s_ps = psum_s.tile([P, NH], f32, tag="sps")
                for c in range(NCH):
                    nc.tensor.matmul(
                        s_ps[:],
                        kt[:, j, c, :],
                        qblk_sb[:, b, c, :],
                        start=(c == 0),
                        stop=(c == NCH - 1),
                    )
                a_j = apool.tile([P, NH], bf16, tag="a")
                nc.scalar.activation(
                    a_j[:],
                    s_ps[:],
                    mybir.ActivationFunctionType.Exp,
                    bias=ebias_sb[:],
                )
                lag.append((j, a_j))
                if len(lag) > 2:
                    _merged(*lag.pop(0))
            while lag:
                _merged(*lag.pop(0))

            # extract the head-diagonal into SBUF: mt_sb[hp*64+p', c, b] =
            # mc[c][hp*64+p', 2c+hp]; split copies across ACT and DVE.
            for c in range(NCH):
                for hp in range(2):
                    h = 2 * c + hp
                    src = mc[c][hp * DH : (hp + 1) * DH, h : h + 1]
                    dst = mt_sb[hp * DH : (hp + 1) * DH, c, b : b + 1]
                    if h % 2 == 0:
                        nc.scalar.copy(dst, src)
                    else:
                        nc.vector.tensor_copy(dst, src)

        # ---- tail: normalize by 1/denom, project through W_o^T
        rall = sp.tile([NH, nb], f32, tag="rall")
        nc.vector.reciprocal(rall[:], dall_ps[:])
        # broadcast rall rows down the partitions via a PE one-hot select:
        # scaleT[p, c, b] = sum_h sel[h, c, p] * rall[h, b] = rall[h(p,c), b]
        scaleT_ps = psum_mcs[0].tile([P, NCH, nb], f32, name="scl", tag="mc")
        for c in range(NCH):
            nc.tensor.matmul(
                scaleT_ps[:, c, :],
                sel_sb[:, c, :],
                rall[:],
                start=True,
                stop=True,
            )
        mt_n = singles.tile([P, NCH, nb], f32)
        nc.vector.tensor_mul(mt_n[:], mt_sb[:], scaleT_ps[:])
        if dbg is not None:
            nc.sync.dma_start(dbg["mt"], mt_sb[:])
            nc.sync.dma_start(dbg["rall"], rall[:])
            scl_sb = singles.tile([P, NCH, nb], f32)
            nc.vector.tensor_copy(scl_sb[:], scaleT_ps[:])
            nc.sync.dma_start(dbg["scl"], scl_sb[:])
        out_ps = psum_o.tile([nb, EMB], f32, tag="ops")
        for c in range(NCH):
            nc.tensor.matmul(
                out_ps[:],
                mt_n[:, c, :],
                wot_sb[:, c, :],
                start=(c == 0),
                stop=(c == NCH - 1),
            )
        out_sb = singles.tile([nb, EMB], f32)
        nc.vector.tensor_copy(out_sb[:], out_ps[:])
        nc.sync.dma_start(out, out_sb[:])


def _prep_all(query, key, value, W_o, mask):
    """Host-side pack: gather unmasked K/V rows, pad, quantize, per-core shards.

    Returns (shards, perm): batches are permuted so every core's positions
    0..NB9-1 hold the batches with >1024 unmasked keys (9 key tiles); the rest
    need only 8. perm[i] = original batch index at packed position i.
    """
    import ml_dtypes

    bf16 = ml_dtypes.bfloat16
    f8e3 = ml_dtypes.float8_e3m4
    q_all = np.ascontiguousarray(query[:, 0, :], dtype=np.float32)  # [B, E]
    keep_all = ~mask[:, 0, :]  # True = attended
    counts_all = keep_all.sum(1).astype(np.int64)
    assert counts_all.max() <= KPAD, f"KPAD too small: {counts_all.max()}"

    heavy = np.flatnonzero(counts_all > (NT - 1) * P)
    light = np.flatnonzero(counts_all <= (NT - 1) * P)
    assert len(heavy) <= N_CORES * NB9, f"too many 9-tile batches: {len(heavy)}"
    pool9 = list(heavy)
    pool8 = list(light)
    per_core = [[] for _ in range(N_CORES)]
    for c in range(N_CORES):
        for _ in range(NB9):
            per_core[c].append(pool9.pop(0) if pool9 else pool8.pop(0))
    for c in range(N_CORES):
        for _ in range(B_LOC - NB9):
            per_core[c].append(pool8.pop(0))
    assert not pool9 and not pool8
    perm = np.array([b for core in per_core for b in core], dtype=np.int64)

    q = q_all[perm]
    keep = keep_all[perm]
    counts = counts_all[perm]

    gidx = np.empty((BATCH, KPAD), dtype=np.int64)
    for i in range(BATCH):
        idx = np.flatnonzero(keep[i])
        n = len(idx)
        gidx[i, :n] = idx
        gidx[i, n:] = idx[0] if n else 0
    rows = (gidx + perm[:, None] * NKEYS).reshape(-1)

    # K: gather packed rows, poison the padded slots, bf16, transpose to
    # [P, B, NCH, KPAD] with kpt[p, b, c, j] = Kp[b, j, c*128+p].
    Kp = key.reshape(-1, EMB)[rows].reshape(BATCH, KPAD, EMB)
    qh2 = (q.reshape(BATCH, NH, DH) ** 2).sum(-1)  # [B, H] per-head |q|^2
    poison = (POISON_S / QSCALE) * q / np.repeat(qh2, DH, axis=1)  # [B, E]
    padpos = np.arange(KPAD)[None, :] >= counts[:, None]
    bi, ji = np.nonzero(padpos)
    Kp[bi, ji] = poison[bi]
    # tile-major: kpt[p, b, t, c, j2] = Kp[b, t*128+j2, c*128+p] so any
    # leading-tile slice is one contiguous per-partition DMA.
    kpt = np.ascontiguousarray(
        Kp.astype(bf16).reshape(BATCH, NT, P, NCH, P).transpose(4, 0, 1, 3, 2)
    )

    # V: gather packed rows (pad rows harmless: their weight is exp(-30)),
    # fp8 e3m4 (max 15.5 >> |V|max ~5.5; rel err ~1.8% washes out in the
    # weighted sum), [P, B, NT, EMB] with vp[p, b, t, e] = Vp[b, t*128+p, e].
    Vp = value.reshape(-1, EMB)[rows].reshape(BATCH, KPAD, EMB)
    vp = np.ascontiguousarray(
        Vp.astype(f8e3).reshape(BATCH, NT, P, EMB).transpose(2, 0, 1, 3)
    )

    # qblk [P, B, NCH, NH]: block-diagonal scaled q. Column h = 2c + (p>=64)
    # holds q[c*128+p]/8; all other columns zero.
    qs = (q * np.float32(QSCALE)).reshape(BATCH, NCH, 2, DH)
    qblk = np.zeros((BATCH, NCH, P, NH), np.float32)
    for c in range(NCH):
        for hp in range(2):
            qblk[:, c, hp * DH : (hp + 1) * DH, 2 * c + hp] = qs[:, c, hp, :]
    qblk = np.ascontiguousarray(qblk.transpose(2, 0, 1, 3).astype(bf16))

    wot = np.ascontiguousarray(W_o.T).astype(np.float32)
    ones = np.ones((P, 1), dtype=bf16)
    sel = np.zeros((NH, NCH, P), dtype=np.float32)
    for c in range(NCH):
        for hp in range(2):
            sel[2 * c + hp, c, hp * DH : (hp + 1) * DH] = 1.0

    shards = []
    for c in range(N_CORES):
        lo, hi = c * B_LOC, (c + 1) * B_LOC
        shards.append(
            {
                "kpt": np.ascontiguousarray(kpt[:, lo:hi]),
                "vp": np.ascontiguousarray(vp[:, lo:hi]),
                "qblk": np.ascontiguousarray(qblk[:, lo:hi]),
                "wot": wot,
                "ones": ones,
                "sel": sel,
            }
        )
    return shards, perm


_NC_CACHE = {}


def _get_nc():
    if "nc" not in _NC_CACHE:
        _NC_CACHE["nc"] = build_nc()
    return _NC_CACHE["nc"]


def kernel(query, key, value, W_o, mask):
    from concourse import bass_utils

    query = np.asarray(query, dtype=np.float32)
    key = np.asarray(key, dtype=np.float32)
    value = np.asarray(value, dtype=np.float32)
    W_o = np.asarray(W_o, dtype=np.float32)
    mask = np.asarray(mask)

    nc = _get_nc()
    in_maps, perm = _prep_all(query, key, value, W_o, mask)
    res = bass_utils.run_bass_kernel_spmd(
        nc, in_maps, core_ids=list(range(N_CORES)), trace=False
    )
    out_perm = np.concatenate([res.results[c]["out"] for c in range(N_CORES)], axis=0)
    out = np.empty_like(out_perm)
    out[perm] = out_perm
    return out.reshape(BATCH, 1, EMB).astype(np.float32, copy=False)


if __name__ == "__main__":
    # smoke: build the program only
    nc = build_nc()
    print("built + compiled OK; instructions:", len(list(nc.all_instructions())))
